# revision 1
# baseline (speedup 1.0000x reference)
"""Distributed kNN retrieval kernel for Trainium2 (8 NeuronCores).

Strategy (pool-sharded, per the standard distributed kNN pattern):
  - The 200000-row embedding pool is split row-wise into 8 shards of 25000
    (zero-padded to 25088 = 49 chunks of 512) — one shard per NeuronCore.
  - Each core computes scores = queries @ shard.T with full-rate bf16
    matmuls (fp32 accumulate), K=1024 accumulated over 8
    PSUM passes, and selects the top-8 scores per 1024-wide slice per query
    on the vector engine (Max + MaxIndex): 25*8 = 200 candidates per
    (query, shard) — a superset of any per-shard top-~160 unless a single
    slice holds >8 of them (verified on the data; Poisson tail ~1e-7).
  - The host merges 8*392 = 3136 candidates per query, takes the top 160
    by device score (bf16 noise ~1e-3 vs a >3e-2 rank-margin), re-scores
    them with an exact software emulation of XLA:CPU's f32 dot kernel
    (two sequential-FMA chunks of 512), sorts, takes top-128, gathers the
    embedding rows and applies the k_predicted mask.

The host re-scoring makes the final ordering bit-identical to the
reference's jnp.dot scores, so the output matches the reference exactly
(up to genuinely tied scores, which are tie-broken by index as lax.top_k
does).
"""

import numpy as np

POOL = 200000
D = 1024
MAXK = 128
NQ = 1024
NSH = 8            # shards / cores
SHW = 25000        # real rows per shard
SHP = 25088        # padded rows per shard (49 * 512)
NCH = 49           # 512-wide chunks per shard
SL = 512           # chunk width == PSUM bank == max fp32 moving operand
NSL = 25           # selection slices: 24 of width 1024 + 1 of width 512
KCH = 8            # contraction chunks (1024 / 128)
NB = 8             # query batches (1024 / 128)
TOPC = 160         # candidates re-scored exactly per query

_cache = {}


def _build():
    import concourse.tile as tile
    from concourse import bacc, mybir
    from contextlib import ExitStack

    nc = bacc.Bacc("TRN2", target_bir_lowering=False, debug=False)
    qT = nc.dram_tensor("qT", [D, NQ], mybir.dt.bfloat16, kind="ExternalInput").ap()
    embT = nc.dram_tensor("embT", [D, SHP], mybir.dt.bfloat16, kind="ExternalInput").ap()
    cand_v = nc.dram_tensor("cand_v", [NQ, NSL * 8], mybir.dt.float32, kind="ExternalOutput").ap()
    cand_i = nc.dram_tensor("cand_i", [NQ, NSL * 8], mybir.dt.uint32, kind="ExternalOutput").ap()

    with tile.TileContext(nc) as tc:
        with ExitStack() as ctx:
            qpool = ctx.enter_context(tc.tile_pool(name="q", bufs=1))
            epool = ctx.enter_context(tc.tile_pool(name="e", bufs=24))
            spool = ctx.enter_context(tc.tile_pool(name="s", bufs=16))
            cpool = ctx.enter_context(tc.tile_pool(name="c", bufs=1))
            pspool = ctx.enter_context(tc.tile_pool(name="ps", bufs=8, space="PSUM"))

            # resident query tiles: per k-chunk [128, 1024] (all batches)
            qts = []
            for k in range(KCH):
                qt = qpool.tile([128, NQ], mybir.dt.bfloat16, tag=f"qt{k}")
                nc.sync.dma_start(qt[:], qT[k * 128:(k + 1) * 128, :])
                qts.append(qt)

            # per-batch candidate accumulators
            mvt = cpool.tile([128, NB * NSL * 8], mybir.dt.float32, tag="mvt")
            mit = cpool.tile([128, NB * NSL * 8], mybir.dt.uint32, tag="mit")

            # process 1024-wide slices (= 2 chunks, except the 512 tail)
            for sl in range(NSL):
                w = min(2 * SL, SHP - sl * 2 * SL)
                nhalves = w // SL
                ets = []
                for k in range(KCH):
                    et = epool.tile([128, 2 * SL], mybir.dt.bfloat16, tag="et")
                    nc.gpsimd.dma_start(
                        et[:, :w], embT[k * 128:(k + 1) * 128,
                                        sl * 2 * SL:sl * 2 * SL + w])
                    ets.append(et)
                sc_tiles = {}
                for half in range(nhalves):
                    for b in range(NB):
                        ps = pspool.tile([128, SL], mybir.dt.float32)
                        for k in range(KCH):
                            nc.tensor.matmul(
                                ps[:], qts[k][:, b * 128:(b + 1) * 128],
                                ets[k][:, half * SL:(half + 1) * SL],
                                start=(k == 0), stop=(k == KCH - 1),
                            )
                        if half == 0:
                            sct = spool.tile([128, 2 * SL], mybir.dt.float32, tag="sc")
                            sc_tiles[b] = sct
                        sc = sc_tiles[b]
                        nc.scalar.copy(sc[:, half * SL:(half + 1) * SL], ps[:])
                        if half == nhalves - 1:
                            o = (b * NSL + sl) * 8
                            seg = sc[:, :w] if w < 2 * SL else sc[:]
                            nc.vector.max(mvt[:, o:o + 8], seg)
                            nc.vector.max_index(mit[:, o:o + 8], mvt[:, o:o + 8], seg)

            for b in range(NB):
                nc.sync.dma_start(cand_v[b * 128:(b + 1) * 128, :],
                                  mvt[:, b * NSL * 8:(b + 1) * NSL * 8])
                nc.sync.dma_start(cand_i[b * 128:(b + 1) * 128, :],
                                  mit[:, b * NSL * 8:(b + 1) * NSL * 8])
    nc.compile()
    return nc


def _get_nc():
    if "nc" not in _cache:
        _cache["nc"] = _build()
    return _cache["nc"]


def _exact_rescore(q_rows, e_rows):
    """Bit-exact emulation of XLA:CPU f32 dot for K=1024: two sequential-FMA
    chunks of 512 (fp64 products+adds rounded to fp32 each step = fused
    multiply-add up to negligible double-rounding), summed in fp32."""
    a = q_rows.astype(np.float64)
    b = e_rows.astype(np.float64)
    out = np.zeros(len(a), np.float32)
    for c in range(2):
        acc = np.zeros(len(a), np.float32)
        for k in range(c * 512, (c + 1) * 512):
            acc = (a[:, k] * b[:, k] + acc).astype(np.float32)
        out = (out + acc).astype(np.float32)
    return out


def _install_ntff_hook():
    """The image's antenv lacks axon_hooks; synthesize it so trace=True works."""
    import sys, types
    if "antenv.axon_hooks" in sys.modules:
        return
    try:
        from trn_agent_boot.trn_boot import _ntff_profile_via_ctypes
        hook = _ntff_profile_via_ctypes("/opt/axon/libaxon_pjrt.so")
    except Exception:
        hook = None
    mod = types.ModuleType("antenv.axon_hooks")
    mod._hook = hook
    mod.get_axon_ntff_profile_hook = lambda: mod._hook
    mod.set_axon_ntff_profile_hook = lambda h: setattr(mod, "_hook", h)
    sys.modules["antenv.axon_hooks"] = mod


def _run_device(qT, shards, trace=False, tmpdir=None):
    import time
    from concourse.bass_utils import run_bass_kernel_spmd
    if trace:
        _install_ntff_hook()
    nc = _get_nc()
    in_maps = [{"qT": qT, "embT": shT} for shT in shards]
    last = None
    for attempt in range(3):
        try:
            return run_bass_kernel_spmd(nc, in_maps, list(range(NSH)), trace=trace, tmpdir=tmpdir)
        except Exception as e:  # transient device wedge: back off and retry
            last = e
            time.sleep(5 * (attempt + 1))
    raise last


def kernel(query_hidden, embeddings, k_predicted, phase_idx=None, _trace=False, _tmpdir=None):
    batch, seq, dim = query_hidden.shape
    q = np.ascontiguousarray(np.asarray(query_hidden, dtype=np.float32).reshape(-1, dim))
    emb = np.ascontiguousarray(np.asarray(embeddings, dtype=np.float32))
    nq = q.shape[0]
    assert (nq, dim) == (NQ, D) and emb.shape == (POOL, D)

    import ml_dtypes
    bf16 = np.dtype(ml_dtypes.bfloat16)
    qT = np.ascontiguousarray(q.T).astype(bf16)
    shards = []
    for s in range(NSH):
        shT = np.zeros((D, SHP), bf16)
        shT[:, :SHW] = emb[s * SHW:(s + 1) * SHW].T.astype(bf16)
        shards.append(shT)

    res = _run_device(qT, shards, trace=_trace, tmpdir=_tmpdir)
    _cache["last_res"] = res

    vals = np.stack([res.results[s]["cand_v"] for s in range(NSH)], 0)  # [8, NQ, 200]
    idxs = np.stack([res.results[s]["cand_i"] for s in range(NSH)], 0)  # [8, NQ, 200]

    # local position -> global pool row (slice s covers [s*1024, s*1024+|s|))
    sl_base = np.arange(NSL * 8, dtype=np.int64) // 8 * (2 * SL)        # [200]
    gidx = (np.arange(NSH, dtype=np.int64)[:, None, None] * SHW
            + sl_base[None, None, :] + idxs.astype(np.int64))           # [8, NQ, 200]
    vals = np.transpose(vals, (1, 0, 2)).reshape(NQ, -1)                # [NQ, 1600]
    gidx = np.transpose(gidx, (1, 0, 2)).reshape(NQ, -1)
    # drop padding hits (score 0 on zero rows can only appear deep below top-160)
    bad = gidx >= POOL
    vals = np.where(bad, -np.inf, vals)

    # top-TOPC by device score per query
    part = np.argpartition(-vals, TOPC, axis=1)[:, :TOPC]               # [NQ, TOPC]
    cidx = np.take_along_axis(gidx, part, 1)                            # [NQ, TOPC]

    # exact re-score (bit-identical to the reference's jnp.dot)
    flat_q = np.repeat(np.arange(NQ), TOPC)
    flat_e = cidx.reshape(-1)
    exact = np.empty(NQ * TOPC, np.float32)
    CH = 262144
    for o in range(0, NQ * TOPC, CH):
        exact[o:o + CH] = _exact_rescore(q[flat_q[o:o + CH]], emb[flat_e[o:o + CH]])
    exact = exact.reshape(NQ, TOPC)

    # reference ordering: descending score, ties -> lower index first
    order = np.lexsort((cidx, -exact.astype(np.float64)), axis=1)[:, :MAXK]
    top_idx = np.take_along_axis(cidx, order, 1)                        # [NQ, 128]

    kp = np.asarray(k_predicted).reshape(-1)
    mask = (np.arange(MAXK)[None, :] < kp[:, None]).astype(np.float32)
    out = emb[top_idx] * mask[:, :, None]
    return out.reshape(batch, seq, MAXK, dim).astype(np.float32)



# revision 6
# speedup vs baseline: 1.9315x; 1.9315x over previous
"""Distributed kNN retrieval kernel for Trainium2 (8 NeuronCores).

Strategy (pool-sharded, fp8 DoubleRow matmul + full int8 score shipping):
  - The 200000-row embedding pool is split row-wise into 8 shards of 25000
    (zero-padded to 25088 = 49 chunks of 512) — one shard per NeuronCore.
  - Each core computes scores = queries @ shard.T entirely in fp8 (e4m3)
    with DoubleRow perf mode: K=256 per instruction at 1 cycle/row — 2x the
    bf16 rate (~157 TF/s). Embeddings are pre-scaled by 64 on the host so
    fp8's normal range covers their 0.02-sigma values; PSUM accumulates in
    f32 so device scores are 64x the true scores plus fp8 input-quantization
    noise (sigma ~0.033 in true-score units).
  - No on-device top-k: every score is converted f32 -> int8 with
    round-to-nearest-even + saturation (scale 28.86 = 127/4.4 in true-score
    units, step 0.035, quantization sigma ~0.010) on the Act and DVE engines
    (split evenly), staged in SBUF, and DMAed to DRAM — 25 MB/core, fully
    overlapped with the matmuls.
  - The host merges the 8 int8 score shards, takes the top-352 candidates
    per query by device score (argpartition; margin analysis: the true
    rank-128..rank-352 score gap is ~4.8 sigma of the combined device noise,
    so the true top-128 is in the candidate set with ~1e-6/query miss odds),
    re-scores them with an exact software emulation of XLA:CPU's f32 dot
    kernel, sorts (ties broken by index as lax.top_k does), takes top-128,
    gathers the embedding rows and applies the k_predicted mask.

The host re-scoring makes the final ordering bit-identical to the
reference's jnp.dot scores, so the output matches the reference exactly
(up to genuinely tied scores).
"""

import numpy as np

POOL = 200000
D = 1024
MAXK = 128
NQ = 1024
NSH = 8            # shards / cores
SHW = 25000        # real rows per shard
SHP = 25088        # padded rows per shard (49 * 512)
KCH = 8            # k chunks of 128 (DoubleRow consumes pairs -> 4 matmuls)
NB = 8             # query batches (1024 / 128)
SL = 512           # psum tile width
NJ = SHP // SL     # 49 512-wide output slices per shard
ESCALE = 64.0      # embedding pre-scale into fp8's normal range
OSCALE = 127.0 / 4.4               # int8 quantization: true score 4.4 -> 127
TOPC = 352         # candidates re-scored exactly per query

_cache = {}


def _build():
    import concourse.tile as tile
    from concourse import bacc, mybir
    from contextlib import ExitStack

    nc = bacc.Bacc("TRN2", target_bir_lowering=False, debug=False)
    q8 = nc.dram_tensor("q8", [128, KCH, NQ], mybir.dt.float8e4, kind="ExternalInput").ap()
    e8 = nc.dram_tensor("e8", [128, KCH, SHP], mybir.dt.float8e4, kind="ExternalInput").ap()
    s8 = nc.dram_tensor("s8", [128, NB, SHP], mybir.dt.int8, kind="ExternalOutput").ap()

    CHW = 2048                     # max positions per e-tile / staging chunk
    # ramped first chunks (compute starts early and the e-DMA queue stays
    # ahead of the PE) and a small tail chunk (final out-DMA drains fast)
    widths = [512, 1024] + [2048] * 11 + [1024]
    assert sum(widths) == SHP

    with tile.TileContext(nc) as tc:
        with ExitStack() as ctx:
            qpool = ctx.enter_context(tc.tile_pool(name="q", bufs=1))
            epool = ctx.enter_context(tc.tile_pool(name="e", bufs=3))
            spool = ctx.enter_context(tc.tile_pool(name="s", bufs=3))
            pspool = ctx.enter_context(tc.tile_pool(name="ps", bufs=8, space="PSUM"))

            # query pair-tiles: first matmul only waits for 256KB, not 1MB
            qts = []
            for i in range(KCH // 2):
                qt = qpool.tile([128, 2, NQ], mybir.dt.float8e4, tag=f"qt{i}")
                nc.sync.dma_start(qt[:], q8[:, 2 * i:2 * i + 2, :])
                qts.append(qt)

            cvt = 0  # alternate converts between Act and DVE
            j0 = 0
            for w in widths:
                nsl = w // SL
                et = epool.tile([128, KCH, CHW], mybir.dt.float8e4, tag="et")
                nc.gpsimd.dma_start(et[:, :, :w], e8[:, :, j0:j0 + w])
                st = spool.tile([128, NB, CHW], mybir.dt.int8, tag="st")
                for jj in range(nsl):
                    for b in range(NB):
                        ps = pspool.tile([128, SL], mybir.dt.float32, tag="ps")
                        for c in range(0, KCH, 2):
                            nc.tensor.matmul(
                                ps[:], qts[c // 2][:, :, b * 128:(b + 1) * 128],
                                et[:, c:c + 2, jj * SL:(jj + 1) * SL],
                                start=(c == 0), stop=(c == KCH - 2),
                                perf_mode=mybir.MatmulPerfMode.DoubleRow,
                            )
                        dst = st[:, b, jj * SL:(jj + 1) * SL]
                        if cvt % 2 == 0:
                            nc.scalar.activation(
                                dst, ps[:], mybir.ActivationFunctionType.Identity,
                                scale=OSCALE / ESCALE)
                        else:
                            nc.vector.tensor_scalar_mul(dst, ps[:], OSCALE / ESCALE)
                        cvt += 1
                nc.sync.dma_start(s8[:, :, j0:j0 + w], st[:, :, :w])
                j0 += w
    nc.compile()
    return nc


def _get_nc():
    if "nc" not in _cache:
        _cache["nc"] = _build()
    return _cache["nc"]


def _exact_rescore(q_rows, e_rows):
    """Bit-exact emulation of XLA:CPU f32 dot for K=1024: two sequential-FMA
    chunks of 512 (fp64 products+adds rounded to fp32 each step = fused
    multiply-add up to negligible double-rounding), summed in fp32."""
    a = q_rows.astype(np.float64)
    b = e_rows.astype(np.float64)
    out = np.zeros(len(a), np.float32)
    for c in range(2):
        acc = np.zeros(len(a), np.float32)
        for k in range(c * 512, (c + 1) * 512):
            acc = (a[:, k] * b[:, k] + acc).astype(np.float32)
        out = (out + acc).astype(np.float32)
    return out


def _install_ntff_hook():
    """The image's antenv lacks axon_hooks; synthesize it so trace=True works."""
    import sys, types
    if "antenv.axon_hooks" in sys.modules:
        return
    try:
        from trn_agent_boot.trn_boot import _ntff_profile_via_ctypes
        hook = _ntff_profile_via_ctypes("/opt/axon/libaxon_pjrt.so")
    except Exception:
        hook = None
    mod = types.ModuleType("antenv.axon_hooks")
    mod._hook = hook
    mod.get_axon_ntff_profile_hook = lambda: mod._hook
    mod.set_axon_ntff_profile_hook = lambda h: setattr(mod, "_hook", h)
    sys.modules["antenv.axon_hooks"] = mod


def _run_device(q8, shards, trace=False, tmpdir=None):
    import time
    from concourse.bass_utils import run_bass_kernel_spmd
    if trace:
        _install_ntff_hook()
    nc = _get_nc()
    in_maps = [{"q8": q8, "e8": sh} for sh in shards]
    last = None
    for attempt in range(3):
        try:
            return run_bass_kernel_spmd(nc, in_maps, list(range(NSH)), trace=trace, tmpdir=tmpdir)
        except Exception as e:  # transient device wedge: back off and retry
            last = e
            time.sleep(5 * (attempt + 1))
    raise last


def _to_dr_layout(mat_fp8, width):
    """[rows, 1024] fp8 -> DoubleRow operand layout [128, 8, width]:
    out[p, c, j] = mat[j, c*128 + p]."""
    t = np.ascontiguousarray(mat_fp8.T)                  # [1024, rows]
    return np.ascontiguousarray(
        t.reshape(KCH, 128, width).transpose(1, 0, 2))   # [128, 8, rows]


def kernel(query_hidden, embeddings, k_predicted, phase_idx=None, _trace=False, _tmpdir=None):
    batch, seq, dim = query_hidden.shape
    q = np.ascontiguousarray(np.asarray(query_hidden, dtype=np.float32).reshape(-1, dim))
    emb = np.ascontiguousarray(np.asarray(embeddings, dtype=np.float32))
    nq = q.shape[0]
    assert (nq, dim) == (NQ, D) and emb.shape == (POOL, D)

    import ml_dtypes
    fp8 = np.dtype(ml_dtypes.float8_e4m3)
    q8 = _to_dr_layout(q.astype(fp8), NQ)
    shards = []
    for s in range(NSH):
        blk = np.zeros((SHP, D), np.float32)
        blk[:SHW] = emb[s * SHW:(s + 1) * SHW]
        shards.append(_to_dr_layout((blk * ESCALE).astype(fp8), SHP))

    res = _run_device(q8, shards, trace=_trace, tmpdir=_tmpdir)
    _cache["last_res"] = res

    # [8, 128, NB, SHP] int8 -> per-core [NQ, SHP] (query = b*128 + p)
    sc = np.empty((NQ, NSH, SHP), np.int8)
    for s in range(NSH):
        sc[:, s, :] = res.results[s]["s8"].transpose(1, 0, 2).reshape(NQ, SHP)
    # widen before negating (-int8(-128) and -int16(-32768) both wrap to
    # themselves and would sort as top); pad value -32767 negates cleanly
    sc = sc.reshape(NQ, NSH * SHP).astype(np.int16)
    # mask padding positions (j >= SHW within each shard)
    pad = (np.arange(NSH * SHP) % SHP) >= SHW
    sc[:, pad] = -32767

    # top-TOPC by device score per query
    part = np.argpartition(-sc, TOPC, axis=1)[:, :TOPC]                 # [NQ, TOPC]
    cidx = (part // SHP) * SHW + (part % SHP)                           # global rows
    assert cidx.max() < POOL, "padding leaked into candidate set"

    # exact re-score (bit-identical to the reference's jnp.dot)
    flat_q = np.repeat(np.arange(NQ), TOPC)
    flat_e = cidx.reshape(-1)
    exact = np.empty(NQ * TOPC, np.float32)
    CH = 262144
    for o in range(0, NQ * TOPC, CH):
        exact[o:o + CH] = _exact_rescore(q[flat_q[o:o + CH]], emb[flat_e[o:o + CH]])
    exact = exact.reshape(NQ, TOPC)

    # reference ordering: descending score, ties -> lower index first
    order = np.lexsort((cidx, -exact.astype(np.float64)), axis=1)[:, :MAXK]
    top_idx = np.take_along_axis(cidx, order, 1)                        # [NQ, 128]

    kp = np.asarray(k_predicted).reshape(-1)
    mask = (np.arange(MAXK)[None, :] < kp[:, None]).astype(np.float32)
    out = emb[top_idx] * mask[:, :, None]
    return out.reshape(batch, seq, MAXK, dim).astype(np.float32)


# revision 11
# speedup vs baseline: 1.9480x; 1.0085x over previous
"""Distributed kNN retrieval kernel for Trainium2 (8 NeuronCores).

Strategy (pool-sharded, fp8 DoubleRow matmul + full int8 score shipping):
  - The 200000-row embedding pool is split row-wise into 8 shards of 25000
    (zero-padded to 25088 = 49 chunks of 512) — one shard per NeuronCore.
  - Each core computes scores = queries @ shard.T entirely in fp8 (e4m3)
    with DoubleRow perf mode: K=256 per instruction at 1 cycle/row — 2x the
    bf16 rate (~157 TF/s). Embeddings are pre-scaled by 64 on the host so
    fp8's normal range covers their 0.02-sigma values; PSUM accumulates in
    f32 so device scores are 64x the true scores plus fp8 input-quantization
    noise (sigma ~0.033 in true-score units).
  - No on-device top-k: every score is converted f32 -> int8 with
    round-to-nearest-even + saturation (scale 28.86 = 127/4.4 in true-score
    units, step 0.035, quantization sigma ~0.010) on the Act and DVE engines
    (split evenly), staged in SBUF, and DMAed to DRAM — 25 MB/core, fully
    overlapped with the matmuls.
  - The host merges the 8 int8 score shards, takes the top-352 candidates
    per query by device score (argpartition; margin analysis: the true
    rank-128..rank-352 score gap is ~4.8 sigma of the combined device noise,
    so the true top-128 is in the candidate set with ~1e-6/query miss odds),
    re-scores them with an exact software emulation of XLA:CPU's f32 dot
    kernel, sorts (ties broken by index as lax.top_k does), takes top-128,
    gathers the embedding rows and applies the k_predicted mask.

The host re-scoring makes the final ordering bit-identical to the
reference's jnp.dot scores, so the output matches the reference exactly
(up to genuinely tied scores).
"""

import numpy as np

POOL = 200000
D = 1024
MAXK = 128
NQ = 1024
NSH = 8            # shards / cores
SHW = 25000        # real rows per shard
SHP = 25088        # padded rows per shard (49 * 512)
KCH = 8            # k chunks of 128 (DoubleRow consumes pairs -> 4 matmuls)
NB = 8             # query batches (1024 / 128)
SL = 512           # psum tile width
NJ = SHP // SL     # 49 512-wide output slices per shard
ESCALE = 64.0      # embedding pre-scale into fp8's normal range
OSCALE = 127.0 / 4.4               # int8 quantization: true score 4.4 -> 127
TOPC = 352         # candidates re-scored exactly per query

_cache = {}


def _build():
    import concourse.tile as tile
    from concourse import bacc, mybir
    from contextlib import ExitStack

    nc = bacc.Bacc("TRN2", target_bir_lowering=False, debug=False)
    q8 = nc.dram_tensor("q8", [128, KCH, NQ], mybir.dt.float8e4, kind="ExternalInput").ap()
    e8 = nc.dram_tensor("e8", [128, KCH, SHP], mybir.dt.float8e4, kind="ExternalInput").ap()
    s8 = nc.dram_tensor("s8", [128, NB, SHP], mybir.dt.int8, kind="ExternalOutput").ap()

    CHW = 2048                     # max positions per e-tile / staging chunk
    # ramped first chunks (compute starts early and the e-DMA queue stays
    # ahead of the PE) and a small tail chunk (final out-DMA drains fast)
    widths = [512, 1024] + [2048] * 11 + [512, 512]
    assert sum(widths) == SHP

    with tile.TileContext(nc) as tc:
        with ExitStack() as ctx:
            qpool = ctx.enter_context(tc.tile_pool(name="q", bufs=1))
            epool = ctx.enter_context(tc.tile_pool(name="e", bufs=3))
            spool = ctx.enter_context(tc.tile_pool(name="s", bufs=3))
            pspool = ctx.enter_context(tc.tile_pool(name="ps", bufs=8, space="PSUM"))

            # query pair-tiles on the (still idle) Act HWDGE queue: first
            # matmul only waits for 256KB, and the sync queue is free for the
            # first e-chunks (the gpsimd software DGE has a ~15us cold start,
            # so bootstrap chunks 0-2 go on sync's hardware DGE instead)
            qts = []
            for i in range(KCH // 2):
                qt = qpool.tile([128, 2, NQ], mybir.dt.float8e4, tag=f"qt{i}")
                nc.scalar.dma_start(qt[:], q8[:, 2 * i:2 * i + 2, :])
                qts.append(qt)

            cvt = 0  # alternate converts between Act and DVE
            j0 = 0
            for ci, w in enumerate(widths):
                nsl = w // SL
                et = epool.tile([128, KCH, CHW], mybir.dt.float8e4, tag="et")
                eng = nc.sync if ci < 3 else nc.gpsimd
                eng.dma_start(et[:, :, :w], e8[:, :, j0:j0 + w])
                st = spool.tile([128, NB, CHW], mybir.dt.int8, tag="st")
                for jj in range(nsl):
                    for b in range(NB):
                        ps = pspool.tile([128, SL], mybir.dt.float32, tag="ps")
                        for c in range(0, KCH, 2):
                            nc.tensor.matmul(
                                ps[:], qts[c // 2][:, :, b * 128:(b + 1) * 128],
                                et[:, c:c + 2, jj * SL:(jj + 1) * SL],
                                start=(c == 0), stop=(c == KCH - 2),
                                perf_mode=mybir.MatmulPerfMode.DoubleRow,
                            )
                        dst = st[:, b, jj * SL:(jj + 1) * SL]
                        if cvt % 2 == 0:
                            nc.scalar.activation(
                                dst, ps[:], mybir.ActivationFunctionType.Identity,
                                scale=OSCALE / ESCALE)
                        else:
                            nc.vector.tensor_scalar_mul(dst, ps[:], OSCALE / ESCALE)
                        cvt += 1
                nc.sync.dma_start(s8[:, :, j0:j0 + w], st[:, :, :w])
                j0 += w
    nc.compile()
    return nc


def _get_nc():
    if "nc" not in _cache:
        _cache["nc"] = _build()
    return _cache["nc"]


def _exact_rescore(qT64, embT64, flat_q, flat_e):
    """Bit-exact emulation of XLA:CPU f32 dot for K=1024: two sequential-FMA
    chunks of 512 (fp64 products+adds rounded to fp32 each step = fused
    multiply-add up to negligible double-rounding), summed in fp32.
    k-major gathers keep each 1.6MB embT64 row cache-resident."""
    out = np.zeros(len(flat_q), np.float32)
    for c in range(2):
        acc = np.zeros(len(flat_q), np.float32)
        for k in range(c * 512, (c + 1) * 512):
            acc = (qT64[k, flat_q] * embT64[k, flat_e] + acc).astype(np.float32)
        out = (out + acc).astype(np.float32)
    return out


def _install_ntff_hook():
    """The image's antenv lacks axon_hooks; synthesize it so trace=True works."""
    import sys, types
    if "antenv.axon_hooks" in sys.modules:
        return
    try:
        from trn_agent_boot.trn_boot import _ntff_profile_via_ctypes
        hook = _ntff_profile_via_ctypes("/opt/axon/libaxon_pjrt.so")
    except Exception:
        hook = None
    mod = types.ModuleType("antenv.axon_hooks")
    mod._hook = hook
    mod.get_axon_ntff_profile_hook = lambda: mod._hook
    mod.set_axon_ntff_profile_hook = lambda h: setattr(mod, "_hook", h)
    sys.modules["antenv.axon_hooks"] = mod


def _run_device(q8, shards, trace=False, tmpdir=None):
    import time
    from concourse.bass_utils import run_bass_kernel_spmd
    if trace:
        _install_ntff_hook()
    nc = _get_nc()
    in_maps = [{"q8": q8, "e8": sh} for sh in shards]
    last = None
    for attempt in range(3):
        try:
            return run_bass_kernel_spmd(nc, in_maps, list(range(NSH)), trace=trace, tmpdir=tmpdir)
        except Exception as e:  # transient device wedge: back off and retry
            last = e
            time.sleep(5 * (attempt + 1))
    raise last


def _to_dr_layout(mat_fp8, width):
    """[rows, 1024] fp8 -> DoubleRow operand layout [128, 8, width]:
    out[p, c, j] = mat[j, c*128 + p]."""
    t = np.ascontiguousarray(mat_fp8.T)                  # [1024, rows]
    return np.ascontiguousarray(
        t.reshape(KCH, 128, width).transpose(1, 0, 2))   # [128, 8, rows]


def kernel(query_hidden, embeddings, k_predicted, phase_idx=None, _trace=False, _tmpdir=None):
    batch, seq, dim = query_hidden.shape
    q = np.ascontiguousarray(np.asarray(query_hidden, dtype=np.float32).reshape(-1, dim))
    emb = np.ascontiguousarray(np.asarray(embeddings, dtype=np.float32))
    nq = q.shape[0]
    assert (nq, dim) == (NQ, D) and emb.shape == (POOL, D)

    import ml_dtypes
    fp8 = np.dtype(ml_dtypes.float8_e4m3)
    q8 = _to_dr_layout(q.astype(fp8), NQ)
    shards = []
    for s in range(NSH):
        blk = np.zeros((SHP, D), np.float32)
        blk[:SHW] = emb[s * SHW:(s + 1) * SHW]
        shards.append(_to_dr_layout((blk * ESCALE).astype(fp8), SHP))

    res = _run_device(q8, shards, trace=_trace, tmpdir=_tmpdir)
    _cache["last_res"] = res

    # [8, 128, NB, SHP] int8 -> per-core [NQ, SHP] (query = b*128 + p)
    sc = np.empty((NQ, NSH, SHP), np.int8)
    for s in range(NSH):
        sc[:, s, :] = res.results[s]["s8"].transpose(1, 0, 2).reshape(NQ, SHP)
    sc = sc.reshape(NQ, NSH * SHP)

    # per-query top->=TOPC threshold via int8 histogram (int8 has 256 values;
    # far cheaper than argpartition on 200M elements). Padding rows (j >= SHW
    # in each shard) score ~0, far below the ~+54 threshold, and are filtered
    # out of the candidate pairs below.
    off = (sc.astype(np.int32) + 128) + (np.arange(NQ, dtype=np.int32)[:, None] << 8)
    hist = np.bincount(off.ravel(), minlength=NQ * 256).reshape(NQ, 256)
    del off
    rev = hist[:, ::-1].cumsum(axis=1)[:, ::-1]      # rev[q,v] = #(sc+128 >= v)
    thr = ((rev >= TOPC).sum(axis=1) - 1 - 128).astype(np.int8)  # per-query
    qi, ci = np.nonzero(sc >= thr[:, None])          # ~TOPC..TOPC+60 per query
    keep = (ci % SHP) < SHW
    qi, ci = qi[keep], ci[keep]
    gidx = (ci // SHP) * SHW + (ci % SHP)            # global pool rows
    assert gidx.max() < POOL
    counts = np.bincount(qi, minlength=NQ)
    assert counts.min() >= MAXK

    # exact re-score (bit-identical to the reference's jnp.dot)
    qT64 = np.ascontiguousarray(q.T).astype(np.float64)
    embT64 = np.ascontiguousarray(emb.T).astype(np.float64)
    exact = _exact_rescore(qT64, embT64, qi, gidx)
    del qT64, embT64

    # reference ordering per query: descending score, ties -> lower index
    order = np.lexsort((gidx, -exact.astype(np.float64), qi))
    starts = np.concatenate(([0], np.cumsum(counts)))[:NQ]
    take = starts[:, None] + np.arange(MAXK)[None, :]
    top_idx = gidx[order][take]                                         # [NQ, 128]

    kp = np.asarray(k_predicted).reshape(-1)
    mask = (np.arange(MAXK)[None, :] < kp[:, None]).astype(np.float32)
    out = emb[top_idx] * mask[:, :, None]
    return out.reshape(batch, seq, MAXK, dim).astype(np.float32)
